# revision 19
# baseline (speedup 1.0000x reference)
"""LSTM discriminator kernel for Trainium2 (8 NeuronCores, SPMD data-parallel).

Problem: B=4096, T=256, D=128, H=32 LSTM + final linear to 2 classes.
Sharding: batch split across 8 cores (512 rows each); weights replicated.

v3 design (2-stream software pipeline, batch-major cell state):
  - Per core: 512 batch = 2 streams x 2 chunks x 128. The two streams are
    independent recurrences whose ops interleave on the engines, hiding the
    per-step dependency-chain latency (the v1 bottleneck).
  - feed uploaded as fp8e4 for the first 240 steps + bf16 for the last 16
    (errors from early steps wash out through the forget gate; halves
    host->device bytes); weights bf16 (mixed-dtype matmul is supported).
  - Per stream step: per chunk one input MM (feed chunk stationary, wihT
    moving, start) + one recurrence MM (stacked h^T [65,128] stationary
    incl. ones bias row, block-diag whh65 [65,128] slice moving, stop) ->
    gates PSUM [128b, 2ch x G] fp32. NOTE: each PSUM region's start/stop
    pair must complete before the next region's start, or has_written
    breaks and the input contribution is dropped.
    Sigmoid ACT over all gates (tanh folded via doubled g-columns); c kept
    as 2*c (c4), h as h/2: DVE t4=f*c4, w=(sig_g-.5)*i, c4=(4w)+t4; ACT
    s=sig(c4); DVE u=(s-.5)*o; PE transposes u -> [64,128] bf16 PSUM; DVE
    copies into HT rows 0:64 (ones row 64 persists).
  - Transpose+copy for stream X are emitted one phase later (during stream
    Y's phase) to avoid head-of-line blocking in the in-order engine queues.
"""

import numpy as np
import ml_dtypes

import concourse.bass as bass
import concourse.mybir as mybir
from concourse.tile import TileContext
from concourse.bass_utils import run_bass_kernel_spmd

F32 = mybir.dt.float32
BF16 = mybir.dt.bfloat16
FP8 = mybir.dt.float8e4
BF = ml_dtypes.bfloat16
F8 = ml_dtypes.float8_e4m3

B, T, D, H = 4096, 256, 128, 32
G = 4 * H          # 128 gate cols (i,f,g,o x 32)
NCORES = 8
BC = B // NCORES   # 512 batch per core
NST = 2            # streams per core
NCH = 2            # chunks of 128 per stream
TCHUNK = 8         # timesteps per feed DMA

SIG = mybir.ActivationFunctionType.Sigmoid
MULT = mybir.AluOpType.mult
SUB = mybir.AluOpType.subtract
ADD = mybir.AluOpType.add

LAST_RESULTS = None  # BassKernelResults of the most recent run (for test.py)

FEED_FP8 = True     # feed in fp8e4 with a bf16 tail (else all bf16)
TAILCH = 2          # trailing TCHUNK-blocks kept in bf16 (16 steps)


# ---------------------------------------------------------------- legalize ---
# This walrus build accepts at most ONE semaphore wait per instruction.
# Hoist excess waits onto injected single-wait NoOps on the same engine.
_lgl_ctr = [0]


def _legalize_sync_waits(nc):
    for fn in nc.m.functions:
        for blk in fn.blocks:
            new = []
            changed = False
            for inst in blk.instructions:
                si = getattr(inst, "sync_info", None)
                waits = list(si.on_wait) if (si is not None and si.on_wait) else []
                if len(waits) > 1:
                    for w in waits[:-1]:
                        _lgl_ctr[0] += 1
                        new.append(mybir.InstNoOp(
                            name=f"I-lgl-{_lgl_ctr[0]}",
                            engine=inst.engine,
                            sync_info=mybir.SyncInfo(on_wait=[w], on_update=[]),
                            bass_nofuse=True,
                        ))
                    si.on_wait = waits[-1:]
                    changed = True
                new.append(inst)
            if changed:
                blk.instructions[:] = new


# ------------------------------------------------------------------ device ---
_nc_cache = None
FEED_DT = FP8 if FEED_FP8 else BF16


def _build_nc(reps=1):
    nc = bass.Bass()

    nf8 = T // TCHUNK - (TAILCH if FEED_FP8 else 0)
    feedT8 = nc.dram_tensor("feedT8", [nf8, D, TCHUNK * BC], FEED_DT,
                            kind="ExternalInput")
    if FEED_FP8:
        feedT8b = nc.dram_tensor("feedT8b", [TAILCH, D, TCHUNK * BC], BF16,
                                 kind="ExternalInput")
    wihT = nc.dram_tensor("wihT", [D, G], BF16, kind="ExternalInput")
    whh65 = nc.dram_tensor("whh65", [H * NCH + 1, NCH * G], BF16,
                           kind="ExternalInput")
    wout65 = nc.dram_tensor("wout65", [H * NCH + 1, NCH * 2], BF16,
                            kind="ExternalInput")
    h0T65 = nc.dram_tensor("h0T65", [H * NCH + 1, NST * 128], BF16,
                           kind="ExternalInput")
    c0q = nc.dram_tensor("c0q", [128, NST * NCH * H], F32, kind="ExternalInput")
    ident_d = nc.dram_tensor("ident_d", [128, 128], BF16, kind="ExternalInput")
    y_out = nc.dram_tensor("y_out", [2, BC], F32, kind="ExternalOutput")

    K65 = H * NCH + 1  # 65

    with TileContext(nc) as tc:
        with (
            tc.tile_pool(name="const", bufs=1) as cpool,
            tc.tile_pool(name="state", bufs=1) as spool,
            tc.tile_pool(name="feed", bufs=2) as fpool,
            tc.tile_pool(name="work", bufs=2) as wpool,
            tc.tile_pool(name="gps", bufs=2, space="PSUM") as gpool,
            tc.tile_pool(name="tps", bufs=2, space="PSUM") as tpool,
            tc.tile_pool(name="yps", bufs=1, space="PSUM") as ypool,
        ):
            wihT_sb = cpool.tile([D, G], BF16, tag="wihT")
            whh_sb = cpool.tile([K65, NCH * G], BF16, tag="whh65")
            wout_sb = cpool.tile([K65, NCH * 2], BF16, tag="wout65")
            ident = cpool.tile([128, 128], BF16, tag="ident")
            HT = [spool.tile([K65, 128], BF16, tag=f"HT{x}", name=f"HT{x}")
                  for x in range(NST)]
            C4 = [spool.tile([128, NCH * H], F32, tag=f"C4{x}", name=f"C4{x}")
                  for x in range(NST)]

            nc.sync.dma_start(wihT_sb[:], wihT[:])
            nc.sync.dma_start(whh_sb[:], whh65[:])
            nc.sync.dma_start(wout_sb[:], wout65[:])
            nc.sync.dma_start(ident[:], ident_d[:])
            for x in range(NST):
                nc.sync.dma_start(HT[x][:], h0T65[:, x * 128:(x + 1) * 128])
                nc.sync.dma_start(C4[x][:], c0q[:, x * NCH * H:(x + 1) * NCH * H])

            def fetch(tb):
                if FEED_FP8 and tb >= nf8:
                    fb = fpool.tile([D, TCHUNK * BC], BF16,
                                    tag="fbufb", name="fbufb")
                    nc.sync.dma_start(fb[:], feedT8b[tb - nf8])
                else:
                    fb = fpool.tile([D, TCHUNK * BC], FEED_DT,
                                    tag="fbuf", name="fbuf")
                    nc.sync.dma_start(fb[:], feedT8[tb])
                return fb

            nchunks = T // TCHUNK
            pending = None  # (stream, u_tile) awaiting transpose+copy
            nxt = fetch(0)
            for rr in range(reps):
              for t in range(T):
                tb, ts = divmod(t, TCHUNK)
                for x in range(NST):
                    if x == 0 and ts == 0:
                        fbuf = nxt
                        # prefetch the next chunk one DMA interval ahead
                        nxt_tb = (tb + 1) if (tb + 1 < nchunks) else (
                            0 if rr + 1 < reps else None)
                        if nxt_tb is not None:
                            nxt = fetch(nxt_tb)

                    # --- deferred tail for the other stream: hT = (sT-.5)*oT
                    # (phase head: s/oT finished last phase, so no PE stall) ---
                    if pending is not None:
                        px, ps, poT = pending
                        sT = tpool.tile([NCH * H, 128], BF16, tag="sT")
                        nc.tensor.transpose(sT[:], ps[:], ident[:])
                        nc.vector.scalar_tensor_tensor(
                            HT[px][0:NCH * H, :], sT[:], 0.5, poT[:], SUB, MULT)
                        pending = None

                    # --- PE: gates = feed @ wihT + HT.T @ whh (+bias row) ---
                    gates = gpool.tile([128, NCH * G], F32, tag="gates")
                    for c in range(NCH):
                        fsl = fbuf[:, ts * BC + (x * NCH + c) * 128:
                                   ts * BC + (x * NCH + c + 1) * 128]
                        nc.tensor.matmul(gates[:, c * G:(c + 1) * G], fsl,
                                         wihT_sb[:], start=True, stop=False)
                        nc.tensor.matmul(gates[:, c * G:(c + 1) * G], HT[x][:],
                                         whh_sb[:, c * G:(c + 1) * G],
                                         start=False, stop=True)

                    # --- ACT: sigmoid over all gates ---
                    acts = wpool.tile([128, NCH * G], BF16, tag="acts")
                    nc.scalar.activation(acts[:], gates[:], SIG)

                    av = acts[:].rearrange("p (c g) -> p c g", c=NCH)
                    i_sl = av[:, :, 0:32]
                    f_sl = av[:, :, 32:64]
                    g_sl = av[:, :, 64:96]
                    o_sl = av[:, :, 96:128]
                    cv = C4[x][:].rearrange("p (c h) -> p c h", c=NCH)

                    # --- PE+ACT (off-chain): oT = transpose(o) -> SBUF ---
                    oT_ps = tpool.tile([NCH * H, 128], BF16, tag="oTp")
                    for c in range(NCH):
                        nc.tensor.transpose(oT_ps[c * H:(c + 1) * H, :],
                                            acts[:, c * G + 96:c * G + 128],
                                            ident[:])
                    oT = wpool.tile([NCH * H, 128], BF16, tag="oT")

                    # --- DVE: t4 = f * c4_old ; w = (sig_g - .5) * i ---
                    t4 = wpool.tile([128, NCH * H], F32, tag="t4")
                    t4v = t4[:].rearrange("p (c h) -> p c h", c=NCH)
                    nc.vector.tensor_tensor(t4v, f_sl, cv, MULT)
                    wt = wpool.tile([128, NCH * H], BF16, tag="wt")
                    wtv = wt[:].rearrange("p (c h) -> p c h", c=NCH)
                    nc.vector.scalar_tensor_tensor(wtv, g_sl, 0.5, i_sl, SUB, MULT)

                    # --- DVE: c4 = 4*w + t4 (in place) ---
                    nc.vector.scalar_tensor_tensor(C4[x][:], wt[:], 4.0, t4[:],
                                                   MULT, ADD)

                    # --- ACT: oT copy fills the a1->a2 gap; s = sigmoid(c4) ---
                    nc.scalar.copy(oT[:], oT_ps[:])
                    s = wpool.tile([128, NCH * H], BF16, tag="s")
                    nc.scalar.activation(s[:], C4[x][:], SIG)

                    pending = (x, s, oT)

            # flush the last stream's state update
            px, ps, poT = pending
            sT = tpool.tile([NCH * H, 128], BF16, tag="sT")
            nc.tensor.transpose(sT[:], ps[:], ident[:])
            nc.vector.scalar_tensor_tensor(
                HT[px][0:NCH * H, :], sT[:], 0.5, poT[:], SUB, MULT)

            # --- final linear: y = 2*hhat @ W_out.T + b_out ---
            y_ps = ypool.tile([2, BC], F32, tag="y")
            for x in range(NST):
                for c in range(NCH):
                    sl = slice((x * NCH + c) * 128, (x * NCH + c + 1) * 128)
                    nc.tensor.matmul(y_ps[:, sl], wout_sb[:, 2 * c:2 * c + 2],
                                     HT[x][:], start=True, stop=True)
            y_sb = wpool.tile([2, BC], F32, tag="ysb")
            nc.scalar.copy(y_sb[:], y_ps[:])
            nc.sync.dma_start(y_out[:], y_sb[:])

    _legalize_sync_waits(nc)
    return nc


# -------------------------------------------------------------------- host ---
def _prep_core_inputs(feed_c, W_ih, W_hh, b_ih, b_hh, W_out, b_out, h0_c, c0_c):
    K65 = H * NCH + 1
    g_rows = slice(64, 96)  # PyTorch gate order i,f,g,o

    wih_p = W_ih.astype(np.float32).copy()
    wih_p[g_rows] *= 2.0
    wihT = np.ascontiguousarray(wih_p.T).astype(BF)

    whh_p = (2.0 * W_hh.astype(np.float32)).copy()
    whh_p[g_rows] *= 2.0
    bias = (b_ih + b_hh).astype(np.float32).copy()
    bias[g_rows] *= 2.0
    whh65 = np.zeros((K65, NCH * G), np.float32)
    for c in range(NCH):
        whh65[32 * c:32 * c + 32, c * G:(c + 1) * G] = whh_p.T
        whh65[64, c * G:(c + 1) * G] = bias
    whh65 = whh65.astype(BF)

    wout65 = np.zeros((K65, NCH * 2), np.float32)
    for c in range(NCH):
        wout65[32 * c:32 * c + 32, 2 * c:2 * c + 2] = 2.0 * W_out.astype(np.float32).T
        wout65[64, 2 * c:2 * c + 2] = b_out
    wout65 = wout65.astype(BF)

    # feed_c [BC, T, D] -> [T, D, BC] -> [T/8, 8, D, BC] -> [T/8, D, 8*BC]
    ft = feed_c.transpose(1, 2, 0).reshape(T // TCHUNK, TCHUNK, D, BC)
    feedall = np.ascontiguousarray(ft.transpose(0, 2, 1, 3)).reshape(
        T // TCHUNK, D, TCHUNK * BC)
    if FEED_FP8:
        nf8 = T // TCHUNK - TAILCH
        feedT8 = feedall[:nf8].astype(F8)
        feedT8b = feedall[nf8:].astype(BF)
    else:
        feedT8 = feedall.astype(BF)

    h0T65 = np.zeros((K65, NST * 128), np.float32)
    h0T65[64] = 1.0
    c0q = np.zeros((128, NST * NCH * H), np.float32)
    for x in range(NST):
        for c in range(NCH):
            rows = slice((x * NCH + c) * 128, (x * NCH + c + 1) * 128)
            h0T65[32 * c:32 * c + 32, x * 128:(x + 1) * 128] = h0_c[rows].T / 2.0
            c0q[:, (x * NCH + c) * H:(x * NCH + c + 1) * H] = 2.0 * c0_c[rows]
    h0T65 = h0T65.astype(BF)
    c0q = c0q.astype(np.float32)

    ident = np.eye(128, dtype=np.float32).astype(BF)

    out = dict(feedT8=feedT8, wihT=wihT, whh65=whh65, wout65=wout65,
               h0T65=h0T65, c0q=c0q, ident_d=ident)
    if FEED_FP8:
        out["feedT8b"] = feedT8b
    return out


def kernel(feed, W_ih, W_hh, b_ih, b_hh, W_out, b_out, h0, c0):
    global _nc_cache, LAST_RESULTS
    feed = np.asarray(feed, dtype=np.float32)
    W_ih = np.asarray(W_ih, dtype=np.float32)
    W_hh = np.asarray(W_hh, dtype=np.float32)
    b_ih = np.asarray(b_ih, dtype=np.float32)
    b_hh = np.asarray(b_hh, dtype=np.float32)
    W_out = np.asarray(W_out, dtype=np.float32)
    b_out = np.asarray(b_out, dtype=np.float32)
    h0 = np.asarray(h0, dtype=np.float32)
    c0 = np.asarray(c0, dtype=np.float32)

    if _nc_cache is None:
        _nc_cache = _build_nc()
    nc = _nc_cache

    in_maps = []
    for c in range(NCORES):
        rows = slice(c * BC, (c + 1) * BC)
        in_maps.append(_prep_core_inputs(
            feed[rows], W_ih, W_hh, b_ih, b_hh, W_out, b_out,
            h0[rows], c0[rows]))

    res = run_bass_kernel_spmd(nc, in_maps, core_ids=list(range(NCORES)))
    LAST_RESULTS = res

    out = np.empty((B, 2), dtype=np.float32)
    for c in range(NCORES):
        out[c * BC:(c + 1) * BC] = res.results[c]["y_out"].T
    return out


# revision 22
# speedup vs baseline: 2.4853x; 2.4853x over previous
"""LSTM discriminator kernel for Trainium2 (8 NeuronCores, SPMD data-parallel).

Problem: B=4096, T=256, D=128, H=32 LSTM + final linear to 2 classes.
Sharding: batch split across 8 cores (512 rows each); weights replicated.

v3 design (2-stream software pipeline, batch-major cell state):
  - Per core: 512 batch = 2 streams x 2 chunks x 128. The two streams are
    independent recurrences whose ops interleave on the engines, hiding the
    per-step dependency-chain latency (the v1 bottleneck).
  - feed uploaded as fp8e4 for the first 240 steps + bf16 for the last 16
    (errors from early steps wash out through the forget gate; halves
    host->device bytes); weights bf16 (mixed-dtype matmul is supported).
  - Per stream step: one whole-width recurrence MM (stacked h^T [65,128]
    stationary incl. ones bias row, block-diag whh65 [65,256] moving,
    start=True) then per chunk one input MM (feed chunk stationary, wihT
    moving, stop=True) -> gates PSUM [128b, 2ch x G] fp32. has_written
    NOTE: never interleave two OPEN accumulation groups in one PSUM tile
    (two starts before their stops silently drops the first group's
    contribution); a single opener + multiple per-region closers is fine.
    Sigmoid ACT over all gates (tanh folded via doubled g-columns); c kept
    as 2*c (c4) in BF16 (cheap DVE 2x packing; ~+6e-4 rel err), h as h/2:
    DVE t4=f*c4, w=(sig_g-.5)*i, c4=(4w)+t4; ACT s=sig(c4). The h-state tail is fused off the chain: o is transposed to
    [64,128] right after the gate sigmoid (PE + ACT copy, off-chain), then
    hT = (sT-.5)*oT is ONE DVE STT reading the s-transpose straight from
    PSUM and writing HT rows 0:64 (ones row 64 persists).
  - Stream X's tail (s-transpose + STT) is emitted at the HEAD of stream
    Y's phase: one phase of slack for s, and it never queues behind Y's
    matmuls in the in-order engine queues. Feed DMA is prefetched one
    8-step chunk ahead. Wall time = T x per-stream chain latency (engines
    are not saturated), so every change above targets chain latency.
"""

import numpy as np
import ml_dtypes

import concourse.bass as bass
import concourse.mybir as mybir
from concourse.tile import TileContext
from concourse.bass_utils import run_bass_kernel_spmd

F32 = mybir.dt.float32
BF16 = mybir.dt.bfloat16
FP8 = mybir.dt.float8e4
BF = ml_dtypes.bfloat16
F8 = ml_dtypes.float8_e4m3

B, T, D, H = 4096, 256, 128, 32
G = 4 * H          # 128 gate cols (i,f,g,o x 32)
NCORES = 8
BC = B // NCORES   # 512 batch per core
NST = 2            # streams per core
NCH = 2            # chunks of 128 per stream
TCHUNK = 8         # timesteps per feed DMA

SIG = mybir.ActivationFunctionType.Sigmoid
MULT = mybir.AluOpType.mult
SUB = mybir.AluOpType.subtract
ADD = mybir.AluOpType.add

LAST_RESULTS = None  # BassKernelResults of the most recent run (for test.py)

FEED_FP8 = True     # feed in fp8e4 with a bf16 tail (else all bf16)
TAILCH = 2          # trailing TCHUNK-blocks kept in bf16 (16 steps)


# ---------------------------------------------------------------- legalize ---
# This walrus build accepts at most ONE semaphore wait per instruction.
# Hoist excess waits onto injected single-wait NoOps on the same engine.
_lgl_ctr = [0]


def _legalize_sync_waits(nc):
    for fn in nc.m.functions:
        for blk in fn.blocks:
            new = []
            changed = False
            for inst in blk.instructions:
                si = getattr(inst, "sync_info", None)
                waits = list(si.on_wait) if (si is not None and si.on_wait) else []
                if len(waits) > 1:
                    for w in waits[:-1]:
                        _lgl_ctr[0] += 1
                        new.append(mybir.InstNoOp(
                            name=f"I-lgl-{_lgl_ctr[0]}",
                            engine=inst.engine,
                            sync_info=mybir.SyncInfo(on_wait=[w], on_update=[]),
                            bass_nofuse=True,
                        ))
                    si.on_wait = waits[-1:]
                    changed = True
                new.append(inst)
            if changed:
                blk.instructions[:] = new


# ------------------------------------------------------------------ device ---
_nc_cache = None
FEED_DT = FP8 if FEED_FP8 else BF16


def _build_nc(reps=1):
    nc = bass.Bass()

    nf8 = T // TCHUNK - (TAILCH if FEED_FP8 else 0)
    feedT8 = nc.dram_tensor("feedT8", [nf8, D, TCHUNK * BC], FEED_DT,
                            kind="ExternalInput")
    if FEED_FP8:
        feedT8b = nc.dram_tensor("feedT8b", [TAILCH, D, TCHUNK * BC], BF16,
                                 kind="ExternalInput")
    wihT = nc.dram_tensor("wihT", [D, G], BF16, kind="ExternalInput")
    whh65 = nc.dram_tensor("whh65", [H * NCH + 1, NCH * G], BF16,
                           kind="ExternalInput")
    wout65 = nc.dram_tensor("wout65", [H * NCH + 1, NCH * 2], BF16,
                            kind="ExternalInput")
    h0T65 = nc.dram_tensor("h0T65", [H * NCH + 1, NST * 128], BF16,
                           kind="ExternalInput")
    c0q = nc.dram_tensor("c0q", [128, NST * NCH * H], BF16, kind="ExternalInput")
    ident_d = nc.dram_tensor("ident_d", [128, 128], BF16, kind="ExternalInput")
    y_out = nc.dram_tensor("y_out", [2, BC], F32, kind="ExternalOutput")

    K65 = H * NCH + 1  # 65

    with TileContext(nc) as tc:
        with (
            tc.tile_pool(name="const", bufs=1) as cpool,
            tc.tile_pool(name="state", bufs=1) as spool,
            tc.tile_pool(name="feed", bufs=2) as fpool,
            tc.tile_pool(name="work", bufs=2) as wpool,
            tc.tile_pool(name="gps", bufs=2, space="PSUM") as gpool,
            tc.tile_pool(name="tps", bufs=2, space="PSUM") as tpool,
            tc.tile_pool(name="yps", bufs=1, space="PSUM") as ypool,
        ):
            wihT_sb = cpool.tile([D, G], BF16, tag="wihT")
            whh_sb = cpool.tile([K65, NCH * G], BF16, tag="whh65")
            wout_sb = cpool.tile([K65, NCH * 2], BF16, tag="wout65")
            ident = cpool.tile([128, 128], BF16, tag="ident")
            HT = [spool.tile([K65, 128], BF16, tag=f"HT{x}", name=f"HT{x}")
                  for x in range(NST)]
            C4 = [spool.tile([128, NCH * H], BF16, tag=f"C4{x}", name=f"C4{x}")
                  for x in range(NST)]

            nc.sync.dma_start(wihT_sb[:], wihT[:])
            nc.sync.dma_start(whh_sb[:], whh65[:])
            nc.sync.dma_start(wout_sb[:], wout65[:])
            nc.sync.dma_start(ident[:], ident_d[:])
            for x in range(NST):
                nc.sync.dma_start(HT[x][:], h0T65[:, x * 128:(x + 1) * 128])
                nc.sync.dma_start(C4[x][:], c0q[:, x * NCH * H:(x + 1) * NCH * H])

            def fetch(tb):
                if FEED_FP8 and tb >= nf8:
                    fb = fpool.tile([D, TCHUNK * BC], BF16,
                                    tag="fbufb", name="fbufb")
                    nc.sync.dma_start(fb[:], feedT8b[tb - nf8])
                else:
                    fb = fpool.tile([D, TCHUNK * BC], FEED_DT,
                                    tag="fbuf", name="fbuf")
                    nc.sync.dma_start(fb[:], feedT8[tb])
                return fb

            nchunks = T // TCHUNK
            pending = None  # (stream, u_tile) awaiting transpose+copy
            nxt = fetch(0)
            for rr in range(reps):
              for t in range(T):
                tb, ts = divmod(t, TCHUNK)
                for x in range(NST):
                    if x == 0 and ts == 0:
                        fbuf = nxt
                        # prefetch the next chunk one DMA interval ahead
                        nxt_tb = (tb + 1) if (tb + 1 < nchunks) else (
                            0 if rr + 1 < reps else None)
                        if nxt_tb is not None:
                            nxt = fetch(nxt_tb)

                    # --- deferred tail for the other stream: hT = (sT-.5)*oT
                    # (phase head: s/oT finished last phase, so no PE stall) ---
                    if pending is not None:
                        px, ps, poT = pending
                        sT = tpool.tile([NCH * H, 128], BF16, tag="sT")
                        nc.tensor.transpose(sT[:], ps[:], ident[:])
                        nc.vector.scalar_tensor_tensor(
                            HT[px][0:NCH * H, :], sT[:], 0.5, poT[:], SUB, MULT)
                        pending = None

                    # --- PE: gates = HT.T @ whh (+bias row) + feed @ wihT.
                    # One whole-width recurrence MM opens the accumulation
                    # (single has_written clear), input MMs close per region.
                    gates = gpool.tile([128, NCH * G], F32, tag="gates")
                    nc.tensor.matmul(gates[:], HT[x][:], whh_sb[:],
                                     start=True, stop=False)
                    for c in range(NCH):
                        fsl = fbuf[:, ts * BC + (x * NCH + c) * 128:
                                   ts * BC + (x * NCH + c + 1) * 128]
                        nc.tensor.matmul(gates[:, c * G:(c + 1) * G], fsl,
                                         wihT_sb[:], start=False, stop=True)

                    # --- ACT: sigmoid over all gates ---
                    acts = wpool.tile([128, NCH * G], BF16, tag="acts")
                    nc.scalar.activation(acts[:], gates[:], SIG)

                    av = acts[:].rearrange("p (c g) -> p c g", c=NCH)
                    i_sl = av[:, :, 0:32]
                    f_sl = av[:, :, 32:64]
                    g_sl = av[:, :, 64:96]
                    o_sl = av[:, :, 96:128]
                    cv = C4[x][:].rearrange("p (c h) -> p c h", c=NCH)

                    # --- PE+ACT (off-chain): oT = transpose(o) -> SBUF ---
                    oT_ps = tpool.tile([NCH * H, 128], BF16, tag="oTp")
                    for c in range(NCH):
                        nc.tensor.transpose(oT_ps[c * H:(c + 1) * H, :],
                                            acts[:, c * G + 96:c * G + 128],
                                            ident[:])
                    oT = wpool.tile([NCH * H, 128], BF16, tag="oT")

                    # --- DVE: t4 = f * c4_old ; w = (sig_g - .5) * i ---
                    t4 = wpool.tile([128, NCH * H], BF16, tag="t4")
                    t4v = t4[:].rearrange("p (c h) -> p c h", c=NCH)
                    nc.vector.tensor_tensor(t4v, f_sl, cv, MULT)
                    wt = wpool.tile([128, NCH * H], BF16, tag="wt")
                    wtv = wt[:].rearrange("p (c h) -> p c h", c=NCH)
                    nc.vector.scalar_tensor_tensor(wtv, g_sl, 0.5, i_sl, SUB, MULT)

                    # --- DVE: c4 = 4*w + t4 (in place) ---
                    nc.vector.scalar_tensor_tensor(C4[x][:], wt[:], 4.0, t4[:],
                                                   MULT, ADD)

                    # --- ACT: oT copy fills the a1->a2 gap; s = sigmoid(c4) ---
                    nc.scalar.copy(oT[:], oT_ps[:])
                    s = wpool.tile([128, NCH * H], BF16, tag="s")
                    nc.scalar.activation(s[:], C4[x][:], SIG)

                    pending = (x, s, oT)

            # flush the last stream's state update
            px, ps, poT = pending
            sT = tpool.tile([NCH * H, 128], BF16, tag="sT")
            nc.tensor.transpose(sT[:], ps[:], ident[:])
            nc.vector.scalar_tensor_tensor(
                HT[px][0:NCH * H, :], sT[:], 0.5, poT[:], SUB, MULT)

            # --- final linear: y = 2*hhat @ W_out.T + b_out ---
            y_ps = ypool.tile([2, BC], F32, tag="y")
            for x in range(NST):
                for c in range(NCH):
                    sl = slice((x * NCH + c) * 128, (x * NCH + c + 1) * 128)
                    nc.tensor.matmul(y_ps[:, sl], wout_sb[:, 2 * c:2 * c + 2],
                                     HT[x][:], start=True, stop=True)
            y_sb = wpool.tile([2, BC], F32, tag="ysb")
            nc.scalar.copy(y_sb[:], y_ps[:])
            nc.sync.dma_start(y_out[:], y_sb[:])

    _legalize_sync_waits(nc)
    return nc


# -------------------------------------------------------------------- host ---
def _prep_core_inputs(feed_c, W_ih, W_hh, b_ih, b_hh, W_out, b_out, h0_c, c0_c):
    K65 = H * NCH + 1
    g_rows = slice(64, 96)  # PyTorch gate order i,f,g,o

    wih_p = W_ih.astype(np.float32).copy()
    wih_p[g_rows] *= 2.0
    wihT = np.ascontiguousarray(wih_p.T).astype(BF)

    whh_p = (2.0 * W_hh.astype(np.float32)).copy()
    whh_p[g_rows] *= 2.0
    bias = (b_ih + b_hh).astype(np.float32).copy()
    bias[g_rows] *= 2.0
    whh65 = np.zeros((K65, NCH * G), np.float32)
    for c in range(NCH):
        whh65[32 * c:32 * c + 32, c * G:(c + 1) * G] = whh_p.T
        whh65[64, c * G:(c + 1) * G] = bias
    whh65 = whh65.astype(BF)

    wout65 = np.zeros((K65, NCH * 2), np.float32)
    for c in range(NCH):
        wout65[32 * c:32 * c + 32, 2 * c:2 * c + 2] = 2.0 * W_out.astype(np.float32).T
        wout65[64, 2 * c:2 * c + 2] = b_out
    wout65 = wout65.astype(BF)

    # feed_c [BC, T, D] -> [T, D, BC] -> [T/8, 8, D, BC] -> [T/8, D, 8*BC]
    ft = feed_c.transpose(1, 2, 0).reshape(T // TCHUNK, TCHUNK, D, BC)
    feedall = np.ascontiguousarray(ft.transpose(0, 2, 1, 3)).reshape(
        T // TCHUNK, D, TCHUNK * BC)
    if FEED_FP8:
        nf8 = T // TCHUNK - TAILCH
        feedT8 = feedall[:nf8].astype(F8)
        feedT8b = feedall[nf8:].astype(BF)
    else:
        feedT8 = feedall.astype(BF)

    h0T65 = np.zeros((K65, NST * 128), np.float32)
    h0T65[64] = 1.0
    c0q = np.zeros((128, NST * NCH * H), np.float32)
    for x in range(NST):
        for c in range(NCH):
            rows = slice((x * NCH + c) * 128, (x * NCH + c + 1) * 128)
            h0T65[32 * c:32 * c + 32, x * 128:(x + 1) * 128] = h0_c[rows].T / 2.0
            c0q[:, (x * NCH + c) * H:(x * NCH + c + 1) * H] = 2.0 * c0_c[rows]
    h0T65 = h0T65.astype(BF)
    c0q = c0q.astype(BF)

    ident = np.eye(128, dtype=np.float32).astype(BF)

    out = dict(feedT8=feedT8, wihT=wihT, whh65=whh65, wout65=wout65,
               h0T65=h0T65, c0q=c0q, ident_d=ident)
    if FEED_FP8:
        out["feedT8b"] = feedT8b
    return out


def kernel(feed, W_ih, W_hh, b_ih, b_hh, W_out, b_out, h0, c0):
    global _nc_cache, LAST_RESULTS
    feed = np.asarray(feed, dtype=np.float32)
    W_ih = np.asarray(W_ih, dtype=np.float32)
    W_hh = np.asarray(W_hh, dtype=np.float32)
    b_ih = np.asarray(b_ih, dtype=np.float32)
    b_hh = np.asarray(b_hh, dtype=np.float32)
    W_out = np.asarray(W_out, dtype=np.float32)
    b_out = np.asarray(b_out, dtype=np.float32)
    h0 = np.asarray(h0, dtype=np.float32)
    c0 = np.asarray(c0, dtype=np.float32)

    if _nc_cache is None:
        _nc_cache = _build_nc()
    nc = _nc_cache

    in_maps = []
    for c in range(NCORES):
        rows = slice(c * BC, (c + 1) * BC)
        in_maps.append(_prep_core_inputs(
            feed[rows], W_ih, W_hh, b_ih, b_hh, W_out, b_out,
            h0[rows], c0[rows]))

    res = run_bass_kernel_spmd(nc, in_maps, core_ids=list(range(NCORES)))
    LAST_RESULTS = res

    out = np.empty((B, 2), dtype=np.float32)
    for c in range(NCORES):
        out[c * BC:(c + 1) * BC] = res.results[c]["y_out"].T
    return out
